# revision 9
# baseline (speedup 1.0000x reference)
"""Trainium2 Bass kernel for DescartesExtension (order-2 polynomial feature map).

reference: out[b, n(i,j)] = x[b,i] * x[b,j] for i<=j in row-major upper-tri order,
x: [256, 1024] f32 -> out: [256, 524800] f32.

Structure used: for fixed i, output columns [off(i), off(i)+D-i) are
x[b,i] * x[b, i:D] -- a per-partition scalar times a contiguous slice
(tensor_scalar_mul on the DVE / activation-with-scale on ACT, batch rows on
partitions).

Sharding (SPMD: one program, 8 cores, per-core differences only in input data):
core c handles segments i = c + 8k, k = 0..127.  Slot k runs a UNIFORM-width op
T_k = 1024 - 8k on a host-shifted input xs_c[b, t] = x[b, t+c] (zero padded), so
every AP in the program is identical across cores.  Core c's slot k therefore
computes its segment (length T_k - c) plus c trailing zeros.  Each core writes a
packed private output [256, 66048]; the host scatters slots back into the full
output and drops the padding tails.

Precision: the tensor operand pipeline runs in bf16 (input converted on the
host, bf16 stores -> half the HBM/fabric store traffic of f32); the
per-partition scalars stay exact f32, packed host-side into a tiny [256,128]
side input (xfs[b,k] = x_shifted[b,8k]).  The host upcasts the gathered
result to f32.  Measured max relative error ~8e-3, under the 2e-2 gate.

Compute is split DVE (tensor_scalar packed 16-bit mode) + ACT (activation
Copy with per-partition f32 scale) so production outruns the ~426 GB/s
store drain and the store queue never starves.
"""

import numpy as np
import ml_dtypes

B = 256
D = 1024
NCORES = 8
NSLOT = D // NCORES  # 128 slots per core
T = [D - NCORES * k for k in range(NSLOT)]  # uniform slot widths 1024, 1016, ..., 8
S = [0] * (NSLOT + 1)  # packed slot offsets
for _k in range(NSLOT):
    S[_k + 1] = S[_k] + T[_k]
OUTW = S[NSLOT]  # 66048 packed columns per core
CHUNK_MAX = 16384  # packed-output SBUF chunk width (32KB/partition bf16)
BUFS = 4  # packed-chunk buffering depth
RAMP = (1, 2, 4)  # slot counts of the pipeline-fill chunks in block 0
TAPER = 4096  # max cols of the final chunks (shortens the tail drain)

_prog_cache = None


def _act_slot(k):
    """Slots handed to the ACT engine (~24% of packed columns)."""
    return k >= 8 and (k - 8) % 4 == 3


def _chunks(ramp, taper=False):
    """Group slots into chunks of <= CHUNK_MAX packed columns.

    `ramp` pre-slices a few tiny chunks at the front so the first store can
    issue almost immediately (pipeline fill), then greedy-packs the rest.
    `taper` re-splits the final chunk into <= TAPER-col pieces so the last
    bytes trickle out with minimal post-compute drain.
    """
    out = []
    k = 0
    for n in ramp:
        e = min(k + n, NSLOT)
        if e > k:
            out.append((k, e, S[k], S[e] - S[k]))
            k = e
    while k < NSLOT:
        e, w = k, 0
        while e < NSLOT and w + T[e] <= CHUNK_MAX:
            w += T[e]
            e += 1
        out.append((k, e, S[k], w))
        k = e
    if taper and out:
        k0, k1, c0, w = out.pop()
        k = k0
        while k < k1:
            e, w = k, 0
            while e < k1 and w + T[e] <= TAPER:
                w += T[e]
                e += 1
            out.append((k, e, S[k], w))
            k = e
    return out


def _build_program():
    global _prog_cache
    if _prog_cache is not None:
        return _prog_cache

    import concourse.bacc as bacc
    import concourse.mybir as mybir
    import concourse.tile as tile

    nc = bacc.Bacc("TRN2", target_bir_lowering=False, debug=False,
                   enable_partition_id=False)
    xs = nc.dram_tensor("xs", [B, D], mybir.dt.bfloat16, kind="ExternalInput").ap()
    xfs = nc.dram_tensor("xfs", [B, NSLOT], mybir.dt.float32,
                         kind="ExternalInput").ap()
    out = nc.dram_tensor("out", [B, OUTW], mybir.dt.bfloat16, kind="ExternalOutput").ap()

    with tile.TileContext(nc) as tc:
        with (
            tc.tile_pool(name="xp", bufs=1) as xp,
            tc.tile_pool(name="op", bufs=BUFS) as op,
        ):
            xb = [xp.tile([128, D], mybir.dt.bfloat16, tag=f"x{b}", name=f"x{b}")
                  for b in range(2)]
            xsc = [xp.tile([128, NSLOT], mybir.dt.float32, tag=f"s{b}", name=f"s{b}")
                   for b in range(2)]
            # Block-0 inputs ride the ACT (scalar) HWDGE ring so the first DVE
            # op unblocks as soon as possible; block-1 inputs queue on sync's
            # ring ahead of the stores.
            nc.scalar.dma_start(xsc[0][:], xfs[0:128, :])
            nc.scalar.dma_start(xb[0][:], xs[0:128, :])
            nc.sync.dma_start(xsc[1][:], xfs[128:256, :])
            nc.sync.dma_start(xb[1][:], xs[128:256, :])
            for blk in range(B // 128):
                for k0, k1, c0, w in _chunks(RAMP if blk == 0 else ()):
                    pt = op.tile([128, CHUNK_MAX], mybir.dt.bfloat16, tag="packed")
                    for k in range(k0, k1):
                        lo = S[k] - c0
                        if _act_slot(k):
                            nc.scalar.mul(
                                pt[:, lo : lo + T[k]],
                                xb[blk][:, NCORES * k : NCORES * k + T[k]],
                                xsc[blk][:, k : k + 1],
                            )
                        else:
                            nc.vector.tensor_scalar_mul(
                                out=pt[:, lo : lo + T[k]],
                                in0=xb[blk][:, NCORES * k : NCORES * k + T[k]],
                                scalar1=xsc[blk][:, k : k + 1],
                            )
                    nc.sync.dma_start(
                        out[blk * 128 : (blk + 1) * 128, c0 : c0 + w], pt[:, :w]
                    )
    nc.compile()
    _prog_cache = nc
    return nc


def _run(x, trace=False, trace_cores=None):
    """Returns (full_output, BassKernelResults)."""
    from concourse.bass_utils import run_bass_kernel_spmd

    x = np.ascontiguousarray(np.asarray(x), dtype=np.float32)
    assert x.shape == (B, D)
    nc = _build_program()

    xbf = x.astype(ml_dtypes.bfloat16)
    in_maps = []
    for c in range(NCORES):
        xsc = np.zeros((B, D), ml_dtypes.bfloat16)
        xsc[:, : D - c] = xbf[:, c:]
        # exact-f32 per-slot scalars: xfs[b, k] = x[b, c + 8k]
        xfc = np.ascontiguousarray(x[:, c::NCORES], np.float32)
        assert xfc.shape == (B, NSLOT)
        in_maps.append({"xs": xsc, "xfs": xfc})

    kw = {}
    if trace:
        kw["trace"] = True
        if trace_cores is not None:
            kw["trace_cores"] = trace_cores
    res = run_bass_kernel_spmd(nc, in_maps, core_ids=list(range(NCORES)), **kw)

    off = np.zeros(D + 1, np.int64)
    off[1:] = np.cumsum(D - np.arange(D))
    full = np.empty((B, D * (D + 1) // 2), np.float32)
    for c in range(NCORES):
        r = res.results[c]["out"]
        for k in range(NSLOT):
            i = c + NCORES * k
            L = D - i
            full[:, off[i] : off[i] + L] = r[:, S[k] : S[k] + L]
    return full, res


def kernel(x):
    return _run(x)[0]


# revision 11
# speedup vs baseline: 1.1466x; 1.1466x over previous
"""Trainium2 Bass kernel for DescartesExtension (order-2 polynomial feature map).

reference: out[b, n(i,j)] = x[b,i] * x[b,j] for i<=j in row-major upper-tri order,
x: [256, 1024] f32 -> out: [256, 524800] f32.

Structure used: for fixed i, output columns [off(i), off(i)+D-i) are
x[b,i] * x[b, i:D] -- a per-partition scalar times a contiguous slice
(tensor_scalar_mul on the DVE / activation-with-scale on ACT, batch rows on
partitions).

Sharding (SPMD: one program, 8 cores, per-core differences only in input data):
core c handles segments i = c + 8k, k = 0..127.  Slot k runs a UNIFORM-width op
T_k = 1024 - 8k on a host-shifted input xs_c[b, t] = x[b, t+c] (zero padded), so
every AP in the program is identical across cores.  Core c's slot k therefore
computes its segment (length T_k - c) plus c trailing zeros.  Each core writes a
packed private output [256, 66048]; the host scatters slots back into the full
output and drops the padding tails.

Precision: the tensor operand pipeline runs in bf16 (input converted on the
host, bf16 stores -> half the HBM/fabric store traffic of f32); the
per-partition scalars stay exact f32, packed host-side into a tiny [256,128]
side input (xfs[b,k] = x_shifted[b,8k]).  The host upcasts the gathered
result to f32.  Measured max relative error ~8e-3, under the 2e-2 gate.

Compute is split DVE (tensor_scalar packed 16-bit mode) + ACT (activation
Copy with per-partition f32 scale) so production outruns the ~426 GB/s
store drain and the store queue never starves.
"""

import numpy as np
import ml_dtypes

B = 256
D = 1024
NCORES = 8
NSLOT = D // NCORES  # 128 slots per core
T = [D - NCORES * k for k in range(NSLOT)]  # uniform slot widths 1024, 1016, ..., 8
S = [0] * (NSLOT + 1)  # packed slot offsets
for _k in range(NSLOT):
    S[_k + 1] = S[_k] + T[_k]
OUTW = S[NSLOT]  # 66048 packed columns per core
CHUNK_MAX = 16384  # packed-output SBUF chunk width (32KB/partition bf16)
BUFS = 4  # packed-chunk buffering depth
RAMP = (1, 2, 4)  # slot counts of the pipeline-fill chunks in block 0
XF_PRE = 256  # f32 scalar-prefix columns loaded before the bulk

_prog_cache = None


def _act_slot(k):
    """Slots handed to the ACT engine (~24% of packed columns)."""
    return k >= 8 and (k - 8) % 4 == 3


def _chunks(ramp, taper=False):
    """Group slots into chunks of <= CHUNK_MAX packed columns.

    `ramp` pre-slices a few tiny chunks at the front so the first store can
    issue almost immediately (pipeline fill), then greedy-packs the rest.
    `taper` re-splits the final chunk into <= TAPER-col pieces so the last
    bytes trickle out with minimal post-compute drain.
    """
    out = []
    k = 0
    for n in ramp:
        e = min(k + n, NSLOT)
        if e > k:
            out.append((k, e, S[k], S[e] - S[k]))
            k = e
    while k < NSLOT:
        e, w = k, 0
        while e < NSLOT and w + T[e] <= CHUNK_MAX:
            w += T[e]
            e += 1
        out.append((k, e, S[k], w))
        k = e
    if taper and out:
        k0, k1, c0, w = out.pop()
        k = k0
        while k < k1:
            e, w = k, 0
            while e < k1 and w + T[e] <= TAPER:
                w += T[e]
                e += 1
            out.append((k, e, S[k], w))
            k = e
    return out


def _build_program():
    global _prog_cache
    if _prog_cache is not None:
        return _prog_cache

    import concourse.bacc as bacc
    import concourse.mybir as mybir
    import concourse.tile as tile

    nc = bacc.Bacc("TRN2", target_bir_lowering=False, debug=False,
                   enable_partition_id=False)
    xs = nc.dram_tensor("xs", [B, D], mybir.dt.bfloat16, kind="ExternalInput").ap()
    xf = nc.dram_tensor("xf", [B, D], mybir.dt.float32, kind="ExternalInput").ap()
    out = nc.dram_tensor("out", [B, OUTW], mybir.dt.bfloat16, kind="ExternalOutput").ap()

    with tile.TileContext(nc) as tc:
        with (
            tc.tile_pool(name="xp", bufs=1) as xp,
            tc.tile_pool(name="op", bufs=BUFS) as op,
        ):
            xb = [xp.tile([128, D], mybir.dt.bfloat16, tag=f"x{b}", name=f"x{b}")
                  for b in range(2)]
            xb32 = [xp.tile([128, D], mybir.dt.float32, tag=f"xf{b}", name=f"xf{b}")
                    for b in range(2)]
            # The two tensors the first DVE op needs go on the ACT (scalar)
            # HWDGE ring; everything else queues on sync's ring ahead of the
            # stores.
            nc.scalar.dma_start(xb[0][:], xs[0:128, :])
            nc.scalar.dma_start(xb32[0][:, :XF_PRE], xf[0:128, :XF_PRE])
            nc.sync.dma_start(xb32[0][:, XF_PRE:], xf[0:128, XF_PRE:])
            nc.sync.dma_start(xb[1][:], xs[128:256, :])
            nc.sync.dma_start(xb32[1][:], xf[128:256, :])
            for blk in range(B // 128):
                for k0, k1, c0, w in _chunks(RAMP if blk == 0 else ()):
                    pt = op.tile([128, CHUNK_MAX], mybir.dt.bfloat16, tag="packed")
                    for k in range(k0, k1):
                        lo = S[k] - c0
                        if _act_slot(k):
                            nc.scalar.mul(
                                pt[:, lo : lo + T[k]],
                                xb[blk][:, NCORES * k : NCORES * k + T[k]],
                                xb32[blk][:, NCORES * k : NCORES * k + 1],
                            )
                        else:
                            nc.vector.tensor_scalar_mul(
                                out=pt[:, lo : lo + T[k]],
                                in0=xb[blk][:, NCORES * k : NCORES * k + T[k]],
                                scalar1=xb32[blk][:, NCORES * k : NCORES * k + 1],
                            )
                    nc.sync.dma_start(
                        out[blk * 128 : (blk + 1) * 128, c0 : c0 + w], pt[:, :w]
                    )
    nc.compile()
    _prog_cache = nc
    return nc


def _run(x, trace=False, trace_cores=None):
    """Returns (full_output, BassKernelResults)."""
    from concourse.bass_utils import run_bass_kernel_spmd

    x = np.ascontiguousarray(np.asarray(x), dtype=np.float32)
    assert x.shape == (B, D)
    nc = _build_program()

    xbf = x.astype(ml_dtypes.bfloat16)
    in_maps = []
    for c in range(NCORES):
        xsc = np.zeros((B, D), ml_dtypes.bfloat16)
        xsc[:, : D - c] = xbf[:, c:]
        xfc = np.zeros((B, D), np.float32)
        xfc[:, : D - c] = x[:, c:]
        in_maps.append({"xs": xsc, "xf": xfc})

    kw = {}
    if trace:
        kw["trace"] = True
        if trace_cores is not None:
            kw["trace_cores"] = trace_cores
    res = run_bass_kernel_spmd(nc, in_maps, core_ids=list(range(NCORES)), **kw)

    off = np.zeros(D + 1, np.int64)
    off[1:] = np.cumsum(D - np.arange(D))
    full = np.empty((B, D * (D + 1) // 2), np.float32)
    for c in range(NCORES):
        r = res.results[c]["out"]
        for k in range(NSLOT):
            i = c + NCORES * k
            L = D - i
            full[:, off[i] : off[i] + L] = r[:, S[k] : S[k] + L]
    return full, res


def kernel(x):
    return _run(x)[0]


# revision 12
# speedup vs baseline: 1.1524x; 1.0050x over previous
"""Trainium2 Bass kernel for DescartesExtension (order-2 polynomial feature map).

reference: out[b, n(i,j)] = x[b,i] * x[b,j] for i<=j in row-major upper-tri order,
x: [256, 1024] f32 -> out: [256, 524800] f32.

Structure used: for fixed i, output columns [off(i), off(i)+D-i) are
x[b,i] * x[b, i:D] -- a per-partition scalar times a contiguous slice
(tensor_scalar_mul on the DVE / activation-with-scale on ACT, batch rows on
partitions).

Sharding (SPMD: one program, 8 cores, per-core differences only in input data):
core c handles segments i = c + 8k, k = 0..127.  Slot k runs a UNIFORM-width op
T_k = 1024 - 8k on a host-shifted input xs_c[b, t] = x[b, t+c] (zero padded), so
every AP in the program is identical across cores.  Core c's slot k therefore
computes its segment (length T_k - c) plus c trailing zeros.  Each core writes a
packed private output [256, 66048]; the host scatters slots back into the full
output and drops the padding tails.

Precision: the tensor operand pipeline runs in bf16 (input converted on the
host, bf16 stores -> half the HBM/fabric store traffic of f32); the
per-partition scalars stay exact f32, packed host-side into a tiny [256,128]
side input (xfs[b,k] = x_shifted[b,8k]).  The host upcasts the gathered
result to f32.  Measured max relative error ~8e-3, under the 2e-2 gate.

Compute is split DVE (tensor_scalar packed 16-bit mode) + ACT (activation
Copy with per-partition f32 scale) so production outruns the ~426 GB/s
store drain and the store queue never starves.
"""

import numpy as np
import ml_dtypes

B = 256
D = 1024
NCORES = 8
NSLOT = D // NCORES  # 128 slots per core
T = [D - NCORES * k for k in range(NSLOT)]  # uniform slot widths 1024, 1016, ..., 8
S = [0] * (NSLOT + 1)  # packed slot offsets
for _k in range(NSLOT):
    S[_k + 1] = S[_k] + T[_k]
OUTW = S[NSLOT]  # 66048 packed columns per core
CHUNK_MAX = 16384  # packed-output SBUF chunk width (32KB/partition bf16)
BUFS = 4  # packed-chunk buffering depth
RAMP = (1, 2, 4, 8, 16)  # slot counts of the pipeline-fill chunks in block 0
TAPER = 4096  # max cols of the final chunks (shortens the tail drain)
XF_PRE = 256  # f32 scalar-prefix columns loaded before the bulk

_prog_cache = None


def _act_slot(k):
    """Slots handed to the ACT engine (~24% of packed columns)."""
    return k >= 8 and (k - 8) % 4 == 3


def _chunks(ramp, taper=False):
    """Group slots into chunks of <= CHUNK_MAX packed columns.

    `ramp` pre-slices a few tiny chunks at the front so the first store can
    issue almost immediately (pipeline fill), then greedy-packs the rest.
    `taper` re-splits the final chunk into <= TAPER-col pieces so the last
    bytes trickle out with minimal post-compute drain.
    """
    out = []
    k = 0
    for n in ramp:
        e = min(k + n, NSLOT)
        if e > k:
            out.append((k, e, S[k], S[e] - S[k]))
            k = e
    while k < NSLOT:
        e, w = k, 0
        while e < NSLOT and w + T[e] <= CHUNK_MAX:
            w += T[e]
            e += 1
        out.append((k, e, S[k], w))
        k = e
    if taper and out:
        k0, k1, c0, w = out.pop()
        k = k0
        while k < k1:
            e, w = k, 0
            while e < k1 and w + T[e] <= TAPER:
                w += T[e]
                e += 1
            out.append((k, e, S[k], w))
            k = e
    return out


def _build_program():
    global _prog_cache
    if _prog_cache is not None:
        return _prog_cache

    import concourse.bacc as bacc
    import concourse.mybir as mybir
    import concourse.tile as tile

    nc = bacc.Bacc("TRN2", target_bir_lowering=False, debug=False,
                   enable_partition_id=False)
    xs = nc.dram_tensor("xs", [B, D], mybir.dt.bfloat16, kind="ExternalInput").ap()
    xf = nc.dram_tensor("xf", [B, D], mybir.dt.float32, kind="ExternalInput").ap()
    out = nc.dram_tensor("out", [B, OUTW], mybir.dt.bfloat16, kind="ExternalOutput").ap()

    with tile.TileContext(nc) as tc:
        with (
            tc.tile_pool(name="xp", bufs=1) as xp,
            tc.tile_pool(name="op", bufs=BUFS) as op,
        ):
            xb = [xp.tile([128, D], mybir.dt.bfloat16, tag=f"x{b}", name=f"x{b}")
                  for b in range(2)]
            xb32 = [xp.tile([128, D], mybir.dt.float32, tag=f"xf{b}", name=f"xf{b}")
                    for b in range(2)]
            # The two tensors the first DVE op needs go on the ACT (scalar)
            # HWDGE ring; everything else queues on sync's ring ahead of the
            # stores.
            nc.scalar.dma_start(xb[0][:], xs[0:128, :])
            nc.scalar.dma_start(xb32[0][:, :XF_PRE], xf[0:128, :XF_PRE])
            nc.sync.dma_start(xb32[0][:, XF_PRE:], xf[0:128, XF_PRE:])
            nc.sync.dma_start(xb[1][:], xs[128:256, :])
            nc.sync.dma_start(xb32[1][:], xf[128:256, :])
            for blk in range(B // 128):
                for k0, k1, c0, w in _chunks(RAMP if blk == 0 else (), taper=blk == 1):
                    pt = op.tile([128, CHUNK_MAX], mybir.dt.bfloat16, tag="packed")
                    for k in range(k0, k1):
                        lo = S[k] - c0
                        if _act_slot(k):
                            nc.scalar.mul(
                                pt[:, lo : lo + T[k]],
                                xb[blk][:, NCORES * k : NCORES * k + T[k]],
                                xb32[blk][:, NCORES * k : NCORES * k + 1],
                            )
                        else:
                            nc.vector.tensor_scalar_mul(
                                out=pt[:, lo : lo + T[k]],
                                in0=xb[blk][:, NCORES * k : NCORES * k + T[k]],
                                scalar1=xb32[blk][:, NCORES * k : NCORES * k + 1],
                            )
                    nc.sync.dma_start(
                        out[blk * 128 : (blk + 1) * 128, c0 : c0 + w], pt[:, :w]
                    )
    nc.compile()
    _prog_cache = nc
    return nc


def _run(x, trace=False, trace_cores=None):
    """Returns (full_output, BassKernelResults)."""
    from concourse.bass_utils import run_bass_kernel_spmd

    x = np.ascontiguousarray(np.asarray(x), dtype=np.float32)
    assert x.shape == (B, D)
    nc = _build_program()

    xbf = x.astype(ml_dtypes.bfloat16)
    in_maps = []
    for c in range(NCORES):
        xsc = np.zeros((B, D), ml_dtypes.bfloat16)
        xsc[:, : D - c] = xbf[:, c:]
        xfc = np.zeros((B, D), np.float32)
        xfc[:, : D - c] = x[:, c:]
        in_maps.append({"xs": xsc, "xf": xfc})

    kw = {}
    if trace:
        kw["trace"] = True
        if trace_cores is not None:
            kw["trace_cores"] = trace_cores
    res = run_bass_kernel_spmd(nc, in_maps, core_ids=list(range(NCORES)), **kw)

    off = np.zeros(D + 1, np.int64)
    off[1:] = np.cumsum(D - np.arange(D))
    full = np.empty((B, D * (D + 1) // 2), np.float32)
    for c in range(NCORES):
        r = res.results[c]["out"]
        for k in range(NSLOT):
            i = c + NCORES * k
            L = D - i
            full[:, off[i] : off[i] + L] = r[:, S[k] : S[k] + L]
    return full, res


def kernel(x):
    return _run(x)[0]
